# revision 3
# baseline (speedup 1.0000x reference)
"""Trainium2 Bass kernel for the CPCA auxiliary loss (nn_CPCA_51754355917033).

Strategy (data-parallel over the env/batch dim n, 16 envs per core):
  - Host side (sharding prep): every gather baked into per-core contiguous
    device inputs -- action-embedding lookup folded through W_ih (with the
    r/z half of b_hh pre-added), h0 gather (fp8), target gather, negative
    gather, not_dones gather.  All matmul operands pre-transposed
    (contraction dim on partitions).
  - Device GRU: r/z input-gate terms are accumulated into PSUM with
    identity-stationary matmuls, so the r/z eviction is a single
    scalar-engine sigmoid straight out of PSUM (1-z uses sigmoid(-x) via
    scale=-1).  Hidden state is kept in fp8 only.
  - Device MLP: preds @ W1a + b1 is computed ONCE (not per block); each of
    the 21 blocks only runs the negs/tg half of L1 and injects the shared
    term at eviction time (vector add + scalar relu-cast).
  - Host combines the 8 cores' (pos_sum, neg_sum, denom) partials.
"""

import numpy as np
import ml_dtypes

import concourse.bass as bass
import concourse.mybir as mybir
import concourse.tile as tile
from concourse import bacc
from concourse import bass_utils

BF16 = ml_dtypes.bfloat16
F8 = ml_dtypes.float8_e4m3
DT = mybir.dt
AF = mybir.ActivationFunctionType
ALU = mybir.AluOpType

N, T, H, K, S, F, EMB, NLOG, NEG = 128, 512, 512, 16, 16, 4, 32, 18, 20
COEFF = 0.1
NC = 8
NPC = N // NC          # envs per core
R = NPC * S            # GRU rows per core (256)
L = T - 1
NBLK = NEG + 1         # 20 negative g-blocks + 1 positive block
BR = F * R             # rows per block (1024)

_PROGRAM_CACHE = {}


# ----------------------------------------------------------------- host prep

def _prep_core(c, inputs, u_list, k_eff):
    acts = np.asarray(inputs["actions"])[..., 0]
    nd = np.asarray(inputs["not_dones"])[..., 0]
    ri = np.asarray(inputs["rnn_inputs"], np.float32)
    ro = np.asarray(inputs["rnn_outputs"], np.float32)
    ti = np.asarray(inputs["time_subsample"]).astype(np.int64)
    neg_idx = np.asarray(inputs["neg_idx"]).astype(np.int64)
    emb_tab = np.asarray(inputs["action_embed"], np.float32)

    ns = slice(c * NPC, (c + 1) * NPC)
    idx = np.arange(k_eff)[:, None] + ti[None, :]          # (k_eff, S)

    # gi = emb @ W_ih.T + b_ih folded on host, with the r/z half of b_hh
    # pre-added (those gate pre-activations go straight into PSUM).
    W_ih = np.asarray(inputs["W_ih"], np.float32)
    b_ih = np.asarray(inputs["b_ih"], np.float32)
    b_hh = np.asarray(inputs["b_hh"], np.float32)
    bias = b_ih.copy()
    bias[:2 * H] += b_hh[:2 * H]
    GIE = np.zeros((NLOG + 1, 1536), np.float32)
    GIE[:NLOG] = emb_tab @ W_ih.T + bias
    GIE[NLOG] = bias
    act_ext = np.full((NPC, L + K), NLOG, np.int64)
    act_ext[:, :L] = acts[ns, :L]
    AI = act_ext[:, idx]                                   # (NPC, k_eff, S)
    gi_all = GIE[AI.transpose(1, 0, 2).reshape(k_eff, R)]  # (k_eff, R, 1536)
    giT = np.ascontiguousarray(
        gi_all.transpose(0, 2, 1).reshape(k_eff, 12, 128, R)
        .transpose(0, 2, 1, 3)).astype(BF16)               # (k_eff,128,12,R)

    H0 = ro[ns][:, ti]                                     # (NPC, S, H)
    h0T = np.ascontiguousarray(
        H0.transpose(2, 0, 1).reshape(4, 128, R)).astype(F8)

    ri_ext = np.zeros((NPC, L + K, H), np.float32)
    ri_ext[:, :L] = ri[ns, 1:]
    idx2 = np.asarray(u_list)[:, None] + ti[None, :]       # (F, S)
    TG = ri_ext[:, idx2]                                   # (NPC, F, S, H)
    tgT = np.ascontiguousarray(
        TG.transpose(3, 1, 0, 2).reshape(H, BR).reshape(4, 128, BR)).astype(F8)

    ni = neg_idx.reshape(F, N, S, NEG)[:, ns]              # (F, NPC, S, NEG)
    P = ni.transpose(3, 0, 1, 2).reshape(-1)               # cols in (g, f, j) order
    negs = ri.reshape(N * T, H)[P]
    negsT = np.ascontiguousarray(negs.T.reshape(4, 128, NEG * BR)).astype(F8)

    nd_ext = np.zeros((NPC, L + K), np.float32)
    nd_ext[:, :L] = nd[ns, :L]
    G = nd_ext[:, idx]                                     # (NPC, k_eff, S)
    ndv = G.transpose(1, 0, 2).reshape(k_eff, R)
    ndvT = np.ascontiguousarray(
        ndv.reshape(k_eff, 2, 128).transpose(2, 0, 1)).astype(np.float32)

    return dict(giT=giT, h0T=h0T, tgT=tgT, negsT=negsT, ndvT=ndvT)


def _prep_weights(inputs):
    W_hh = np.asarray(inputs["W_hh"], np.float32)
    b_hh = np.asarray(inputs["b_hh"], np.float32)
    W1 = np.asarray(inputs["W1"], np.float32)
    b1 = np.asarray(inputs["b1"], np.float32)
    W2 = np.asarray(inputs["W2"], np.float32)
    b2 = np.asarray(inputs["b2"], np.float32)
    W3 = np.asarray(inputs["W3"], np.float32)
    b3 = np.asarray(inputs["b3"], np.float32)

    d = {}
    d["w_hh8"] = np.ascontiguousarray(
        W_hh.T.reshape(2, 2, 128, 1536).transpose(0, 2, 1, 3)).astype(F8)
    def pack8(WT):
        # [t, ki, ko, m] with contract index = t*256 + ko*128 + ki
        return np.ascontiguousarray(
            WT.reshape(2, 2, 128, WT.shape[1]).transpose(0, 2, 1, 3)).astype(F8)
    d["w1a8"] = pack8(W1[:, :512].T.copy())
    d["w1b8"] = pack8(W1[:, 512:].T.copy())
    d["w28"] = pack8(W2.T.copy())
    d["w3T"] = np.ascontiguousarray(W3[0].reshape(4, 128).T).astype(BF16)
    bg = np.zeros((128, 16), np.float32)
    for cc in range(12):
        bg[:, cc] = b_hh[cc * 128:(cc + 1) * 128]
    d["bgates"] = bg
    d["b1T"] = np.ascontiguousarray(b1.reshape(4, 128).T).astype(np.float32)
    d["b2T"] = np.ascontiguousarray(b2.reshape(4, 128).T).astype(np.float32)
    d["idt"] = np.eye(128, dtype=BF16)
    d["b3f"] = float(b3.reshape(-1)[0])
    return d


# ------------------------------------------------------------- device program

def _build_program(u_list, k_eff, b3f):
    nc = bacc.Bacc("TRN2", target_bir_lowering=False, debug=False, num_devices=NC)

    di = {}
    def inp(name, shape, dt):
        di[name] = nc.dram_tensor(name, list(shape), dt, kind="ExternalInput")
        return di[name]

    d_whh = inp("w_hh8", (2, 128, 2, 1536), DT.float8e4)
    d_w1a = inp("w1a8", (2, 128, 2, 512), DT.float8e4)
    d_w1b = inp("w1b8", (2, 128, 2, 512), DT.float8e4)
    d_w2 = inp("w28", (2, 128, 2, 512), DT.float8e4)
    d_w3 = inp("w3T", (128, 4), DT.bfloat16)
    d_bg = inp("bgates", (128, 16), DT.float32)
    d_b1 = inp("b1T", (128, 4), DT.float32)
    d_b2 = inp("b2T", (128, 4), DT.float32)
    d_idt = inp("idt", (128, 128), DT.bfloat16)
    d_gi = inp("giT", (k_eff, 128, 12, R), DT.bfloat16)
    d_h0 = inp("h0T", (4, 128, R), DT.float8e4)
    d_tg = inp("tgT", (4, 128, BR), DT.float8e4)
    d_negs = inp("negsT", (4, 128, NEG * BR), DT.float8e4)
    d_ndv = inp("ndvT", (128, k_eff, 2), DT.float32)
    d_out = nc.dram_tensor("out", [1, 4], DT.float32, kind="ExternalOutput")

    f32 = DT.float32
    bf16 = DT.bfloat16
    f8 = DT.float8e4

    with tile.TileContext(nc) as tc:
        with (
            tc.tile_pool(name="const", bufs=1) as cp,
            tc.tile_pool(name="gruw", bufs=2) as gp,
            tc.tile_pool(name="mlpw", bufs=3) as mp,
            tc.tile_pool(name="psg", bufs=1, space="PSUM") as pg,
            tc.tile_pool(name="psm", bufs=2, space="PSUM") as pm,
        ):
            # ------------------------------------------------ constant loads
            whh = cp.tile([128, 2, 2, 1536], f8, tag="whh")
            for th in range(2):
                nc.sync.dma_start(out=whh[:, th, :, :], in_=d_whh[th])
            w1a = cp.tile([128, 2, 2, 512], f8, tag="w1a")
            w1b = cp.tile([128, 2, 2, 512], f8, tag="w1b")
            w2 = cp.tile([128, 2, 2, 512], f8, tag="w2")
            for (t, d) in ((w1a, d_w1a), (w1b, d_w1b), (w2, d_w2)):
                for th in range(2):
                    nc.sync.dma_start(out=t[:, th, :, :], in_=d[th])
            w3 = cp.tile([128, 4], bf16, tag="w3")
            nc.sync.dma_start(out=w3[:], in_=d_w3[:])
            bg = cp.tile([128, 16], f32, tag="bg")
            nc.sync.dma_start(out=bg[:], in_=d_bg[:])
            b1 = cp.tile([128, 4], f32, tag="b1")
            nc.sync.dma_start(out=b1[:], in_=d_b1[:])
            b2 = cp.tile([128, 4], f32, tag="b2")
            nc.sync.dma_start(out=b2[:], in_=d_b2[:])
            idt = cp.tile([128, 128], bf16, tag="idt")
            nc.sync.dma_start(out=idt[:], in_=d_idt[:])
            tg = cp.tile([128, 4, BR], f8, tag="tg")
            for kc in range(4):
                nc.sync.dma_start(out=tg[:, kc, :], in_=d_tg[kc])
            ndv = cp.tile([128, k_eff, 2], f32, tag="ndv")
            nc.sync.dma_start(out=ndv[:], in_=d_ndv[:])

            # ------------------------------------------------ forward mask
            prod = cp.tile([128, k_eff, 2], f32, tag="prod")
            nc.vector.tensor_scalar(prod[:, 0, :], ndv[:, 0, :], 0.0, None,
                                    op0=ALU.is_gt)
            for k in range(1, k_eff):
                nc.vector.scalar_tensor_tensor(
                    prod[:, k, :], in0=ndv[:, k, :], scalar=0.0,
                    in1=prod[:, k - 1, :], op0=ALU.is_gt, op1=ALU.mult)
            mfT = cp.tile([128, 2 * F], f32, tag="mfT")
            for fi, u in enumerate(u_list):
                nc.vector.tensor_copy(mfT[:, 2 * fi:2 * fi + 2], prod[:, u, :])

            # ------------------------------------------------ GRU
            # PSUM layout per step: one [128, 8, 256] tile; r gates at
            # [:, 0:4], z gates at [:, 4:8]; the g gates reuse [:, 0:4]
            # after the r sigmoid has evicted.  gi(r/z) (with b_ih + b_hh
            # baked in on host) is accumulated by identity-stationary
            # matmuls, so r/z/(1-z) evict as pure scalar-engine sigmoids.
            DRM = mybir.MatmulPerfMode.DoubleRow
            h8_prev = gp.tile([128, 4, R], f8, tag="h8")
            for kc in range(4):
                nc.sync.dma_start(out=h8_prev[:, kc, :], in_=d_h0[kc])
            predsT = cp.tile([128, 4, BR], f8, tag="preds")

            for k in range(k_eff):
                gi = gp.tile([128, 12, R], bf16, tag="gi", bufs=3)
                nc.sync.dma_start(out=gi[:], in_=d_gi[k])
                ps = pg.tile([128, 8, R], f32, tag="prz")
                # identity-accumulate the r/z input-gate terms (4x N=512)
                for q in range(4):
                    nc.tensor.matmul(ps[:, 2 * q:2 * q + 2, :], idt[:],
                                     gi[:, 2 * q:2 * q + 2, :],
                                     start=True, stop=False)
                # r gates then z gates: Whh contributions
                for gc in range(8):
                    for th in range(2):
                        nc.tensor.matmul(
                            ps[:, gc, :], whh[:, th, :, gc * 128:(gc + 1) * 128],
                            h8_prev[:, 2 * th:2 * th + 2, :],
                            start=False, stop=(th == 1), perf_mode=DRM)
                r_sb = gp.tile([128, 4, R], bf16, tag="r")
                z_sb = gp.tile([128, 4, R], bf16, tag="z")
                w1m = gp.tile([128, 4, R], bf16, tag="w1m")
                e_sb = gp.tile([128, 4, R], bf16, tag="e")
                nc.scalar.activation(r_sb[:], ps[:, 0:4, :], AF.Sigmoid)
                nc.scalar.activation(z_sb[:], ps[:, 4:8, :], AF.Sigmoid)
                nc.scalar.activation(w1m[:], ps[:, 4:8, :], AF.Sigmoid,
                                     scale=-1.0)
                nc.vector.tensor_mul(e_sb[:], z_sb[:], h8_prev[:])
                # g gates reuse the r region of the PSUM tile; each gate's
                # t-term evicts as soon as its pair of matmuls lands
                t_sb = gp.tile([128, 4, R], bf16, tag="t", bufs=1)
                for c in range(4):
                    gc = 8 + c
                    for th in range(2):
                        nc.tensor.matmul(
                            ps[:, c, :], whh[:, th, :, gc * 128:(gc + 1) * 128],
                            h8_prev[:, 2 * th:2 * th + 2, :],
                            start=(th == 0), stop=(th == 1), perf_mode=DRM)
                    nc.vector.scalar_tensor_tensor(
                        t_sb[:, c, :], in0=ps[:, c, :],
                        scalar=bg[:, 8 + c:9 + c],
                        in1=r_sb[:, c, :], op0=ALU.add, op1=ALU.mult)
                u_sb = gp.tile([128, 4, R], bf16, tag="u", bufs=1)
                nc.vector.tensor_add(u_sb[:], gi[:, 8:12, :], t_sb[:])
                g_sb = gp.tile([128, 4, R], bf16, tag="g")
                nc.scalar.activation(g_sb[:], u_sb[:], AF.Tanh)
                gw = gp.tile([128, 4, R], bf16, tag="gw", bufs=1)
                nc.vector.tensor_mul(gw[:], g_sb[:], w1m[:])
                h8_new = gp.tile([128, 4, R], f8, tag="h8")
                nc.vector.tensor_add(h8_new[:], gw[:], e_sb[:])
                h8_prev = h8_new
                for fi, u in enumerate(u_list):
                    if u == k:
                        nc.vector.tensor_copy(
                            predsT[:, :, fi * R:(fi + 1) * R], h8_new[:])

            # ------------------------------------- preds @ W1a + b1 (once)
            p1a = cp.tile([128, 4, BR], bf16, tag="p1a")
            for cc in range(4):
                psp = pm.tile([128, 2, 512], f32, tag="pm")
                for rt in range(2):
                    sl = slice(rt * 512, (rt + 1) * 512)
                    for th in range(2):
                        nc.tensor.matmul(
                            psp[:, rt, :],
                            w1a[:, th, :, cc * 128:(cc + 1) * 128],
                            predsT[:, 2 * th:2 * th + 2, sl],
                            start=(th == 0), stop=(th == 1), perf_mode=DRM)
                nc.scalar.activation(p1a[:, cc, :], psp[:], AF.Identity,
                                     bias=b1[:, cc:cc + 1])

            # ------------------------------------------------ blocks
            logits = cp.tile([128, NBLK, 8], f32, tag="logits")
            for b in range(NBLK):
                if b < NEG:
                    xt = mp.tile([128, 4, BR], f8, tag="negsx")
                    for kc in range(4):
                        nc.sync.dma_start(
                            out=xt[:, kc, :],
                            in_=d_negs[kc][:, b * BR:(b + 1) * BR])
                else:
                    xt = tg
                y1 = mp.tile([128, 4, BR], f8, tag="y1", bufs=2)
                for cc in range(4):
                    psb = pm.tile([128, 2, 512], f32, tag="pm")
                    for rt in range(2):
                        sl = slice(rt * 512, (rt + 1) * 512)
                        for th in range(2):
                            nc.tensor.matmul(
                                psb[:, rt, :],
                                w1b[:, th, :, cc * 128:(cc + 1) * 128],
                                xt[:, 2 * th:2 * th + 2, sl],
                                start=(th == 0), stop=(th == 1), perf_mode=DRM)
                    y1t = mp.tile([128, 2, 512], bf16, tag="y1t", bufs=2)
                    nc.vector.tensor_add(y1t[:], psb[:], p1a[:, cc, :])
                    nc.scalar.activation(y1[:, cc, :], y1t[:], AF.Relu)
                y2 = mp.tile([128, 4, BR], bf16, tag="y2", bufs=2)
                for cc in range(4):
                    psb = pm.tile([128, 2, 512], f32, tag="pm")
                    for rt in range(2):
                        sl = slice(rt * 512, (rt + 1) * 512)
                        for th in range(2):
                            nc.tensor.matmul(
                                psb[:, rt, :],
                                w2[:, th, :, cc * 128:(cc + 1) * 128],
                                y1[:, 2 * th:2 * th + 2, sl],
                                start=(th == 0), stop=(th == 1), perf_mode=DRM)
                    nc.scalar.activation(y2[:, cc, :], psb[:], AF.Relu,
                                         bias=b2[:, cc:cc + 1])
                ps3 = pm.tile([128, 2, 512], f32, tag="pm")
                for col in range(8):
                    for kc in range(4):
                        nc.tensor.matmul(
                            ps3[:, 0, col:col + 1],
                            y2[:, kc, col * 128:(col + 1) * 128],
                            w3[:, kc:kc + 1], start=(kc == 0), stop=(kc == 3))
                nc.scalar.activation(logits[:, b, :], ps3[:, 0, 0:8], AF.Copy)

            # ------------------------------------- softplus + sums
            # softplus(t) = relu(t) - ln(sigmoid(|t|)); whole-tensor ACT ops
            # keep the activation-table sequence to a single switch.
            partials = cp.tile([128, NBLK + 1], f32, tag="partials")
            sp_a = cp.tile([128, NBLK, 8], f32, tag="sp_a")
            sp_l = cp.tile([128, NBLK, 8], f32, tag="sp_l")
            sp_r = cp.tile([128, NBLK, 8], f32, tag="sp_r")
            sp_d = cp.tile([128, 8], f32, tag="sp_d")
            nc.scalar.activation(sp_a[:], logits[:], AF.Abs, bias=b3f)
            nc.scalar.activation(sp_a[:], sp_a[:], AF.Sigmoid)
            nc.scalar.activation(sp_l[:], sp_a[:], AF.Ln)
            nc.scalar.activation(sp_r[:, :NEG, :], logits[:, :NEG, :],
                                 AF.Relu, bias=b3f)
            nc.scalar.activation(sp_r[:, NEG, :], logits[:, NEG, :],
                                 AF.Relu, bias=-b3f, scale=-1.0)
            nc.vector.tensor_sub(sp_r[:], sp_r[:], sp_l[:])
            for b in range(NBLK):
                nc.vector.tensor_mul(sp_d[:], sp_r[:, b, :], mfT[:])
                nc.vector.tensor_reduce(partials[:, b:b + 1], sp_d[:],
                                        mybir.AxisListType.X, ALU.add)
            nc.vector.tensor_reduce(partials[:, NBLK:NBLK + 1], mfT[:],
                                    mybir.AxisListType.X, ALU.add)

            vcol = cp.tile([128, 4], f32, tag="vcol")
            nc.vector.tensor_copy(vcol[:, 0:1], partials[:, NEG:NEG + 1])
            nc.vector.tensor_reduce(vcol[:, 1:2], partials[:, 0:NEG],
                                    mybir.AxisListType.X, ALU.add)
            nc.vector.tensor_copy(vcol[:, 2:3], partials[:, NBLK:NBLK + 1])
            nc.any.memset(vcol[:, 3:4], 0.0)
            ones = cp.tile([128, 1], f32, tag="ones")
            nc.any.memset(ones[:], 1.0)
            psf = pm.tile([128, 2, 512], f32, tag="pm")
            nc.tensor.matmul(psf[0:1, 0, 0:4], ones[:], vcol[:],
                             start=True, stop=True)
            out_sb = cp.tile([1, 4], f32, tag="out_sb")
            nc.scalar.activation(out_sb[:], psf[0:1, 0, 0:4], AF.Copy)
            nc.sync.dma_start(out=d_out[:], in_=out_sb[:])

    nc.finalize()
    return nc


def _get_program(u_list, k_eff, b3f):
    key = (tuple(u_list), k_eff, float(b3f))
    if key not in _PROGRAM_CACHE:
        _PROGRAM_CACHE[key] = _build_program(u_list, k_eff, b3f)
    return _PROGRAM_CACHE[key]


# ------------------------------------------------------------------ kernel

def kernel(**inputs):
    u_list = [int(x) for x in np.asarray(inputs["unroll_subsample"]).reshape(-1)]
    k_eff = max(u_list) + 1
    w = _prep_weights(inputs)
    nc = _get_program(u_list, k_eff, w["b3f"])

    wmaps = {k: v for k, v in w.items() if k != "b3f"}
    in_maps = []
    for c in range(NC):
        m = dict(wmaps)
        m.update(_prep_core(c, inputs, u_list, k_eff))
        in_maps.append(m)

    res = bass_utils.run_bass_kernel_spmd(nc, in_maps, list(range(NC)))
    P = Ng = D = 0.0
    for c in range(NC):
        o = np.asarray(res.results[c]["out"], np.float64)
        P += o[0, 0]
        Ng += o[0, 1]
        D += o[0, 2]
    loss = COEFF * (P / D + Ng / (D * NEG))
    return np.float32(loss)


# revision 6
# speedup vs baseline: 1.2623x; 1.2623x over previous
"""Trainium2 Bass kernel for the CPCA auxiliary loss (nn_CPCA_51754355917033).

Strategy (data-parallel over the env/batch dim n, 16 envs per core):
  - Host side (sharding prep): every gather baked into per-core contiguous
    device inputs -- action-embedding lookup folded through W_ih (with the
    r/z half of b_hh pre-added), h0 gather (fp8), target gather, negative
    gather, not_dones gather.  All matmul operands pre-transposed
    (contraction dim on partitions).
  - Device GRU: r/z input-gate terms are accumulated into PSUM with
    identity-stationary matmuls, so the r/z eviction is a single
    scalar-engine sigmoid straight out of PSUM (1-z uses sigmoid(-x) via
    scale=-1).  Hidden state is kept in fp8 only.
  - Device MLP: preds @ W1a + b1 is computed ONCE (not per block); each of
    the 21 blocks only runs the negs/tg half of L1 and injects the shared
    term at eviction time (vector add + scalar relu-cast).
  - Host combines the 8 cores' (pos_sum, neg_sum, denom) partials.
"""

import numpy as np
import ml_dtypes

import concourse.bass as bass
import concourse.mybir as mybir
import concourse.tile as tile
from concourse import bacc
from concourse import bass_utils

BF16 = ml_dtypes.bfloat16
F8 = ml_dtypes.float8_e4m3
DT = mybir.dt
AF = mybir.ActivationFunctionType
ALU = mybir.AluOpType

N, T, H, K, S, F, EMB, NLOG, NEG = 128, 512, 512, 16, 16, 4, 32, 18, 20
COEFF = 0.1
NC = 8
NPC = N // NC          # envs per core
R = NPC * S            # GRU rows per core (256)
L = T - 1
NBLK = NEG + 1         # 20 negative g-blocks + 1 positive block
BR = F * R             # rows per block (1024)

_PROGRAM_CACHE = {}


# ----------------------------------------------------------------- host prep

def _prep_core(c, inputs, u_list, k_eff):
    acts = np.asarray(inputs["actions"])[..., 0]
    nd = np.asarray(inputs["not_dones"])[..., 0]
    ri = np.asarray(inputs["rnn_inputs"], np.float32)
    ro = np.asarray(inputs["rnn_outputs"], np.float32)
    ti = np.asarray(inputs["time_subsample"]).astype(np.int64)
    neg_idx = np.asarray(inputs["neg_idx"]).astype(np.int64)
    emb_tab = np.asarray(inputs["action_embed"], np.float32)

    ns = slice(c * NPC, (c + 1) * NPC)
    idx = np.arange(k_eff)[:, None] + ti[None, :]          # (k_eff, S)

    # gi = emb @ W_ih.T + b_ih folded on host, with the r/z half of b_hh
    # pre-added (those gate pre-activations go straight into PSUM).
    W_ih = np.asarray(inputs["W_ih"], np.float32)
    b_ih = np.asarray(inputs["b_ih"], np.float32)
    b_hh = np.asarray(inputs["b_hh"], np.float32)
    bias = b_ih.copy()
    bias[:2 * H] += b_hh[:2 * H]
    GIE = np.zeros((NLOG + 1, 1536), np.float32)
    GIE[:NLOG] = emb_tab @ W_ih.T + bias
    GIE[NLOG] = bias
    act_ext = np.full((NPC, L + K), NLOG, np.int64)
    act_ext[:, :L] = acts[ns, :L]
    AI = act_ext[:, idx]                                   # (NPC, k_eff, S)
    gi_all = GIE[AI.transpose(1, 0, 2).reshape(k_eff, R)]  # (k_eff, R, 1536)
    giT = np.ascontiguousarray(
        gi_all.transpose(0, 2, 1).reshape(k_eff, 12, 128, R)
        .transpose(0, 2, 1, 3)).astype(BF16)               # (k_eff,128,12,R)

    H0 = ro[ns][:, ti]                                     # (NPC, S, H)
    h0T = np.ascontiguousarray(
        H0.transpose(2, 0, 1).reshape(4, 128, R)).astype(F8)

    ri_ext = np.zeros((NPC, L + K, H), np.float32)
    ri_ext[:, :L] = ri[ns, 1:]
    idx2 = np.asarray(u_list)[:, None] + ti[None, :]       # (F, S)
    TG = ri_ext[:, idx2]                                   # (NPC, F, S, H)
    tgT = np.ascontiguousarray(
        TG.transpose(3, 1, 0, 2).reshape(H, BR).reshape(4, 128, BR)).astype(F8)

    ni = neg_idx.reshape(F, N, S, NEG)[:, ns]              # (F, NPC, S, NEG)
    P = ni.transpose(3, 0, 1, 2).reshape(-1)               # cols in (g, f, j) order
    negs = ri.reshape(N * T, H)[P]
    negsT = np.ascontiguousarray(negs.T.reshape(4, 128, NEG * BR)).astype(F8)

    nd_ext = np.zeros((NPC, L + K), np.float32)
    nd_ext[:, :L] = nd[ns, :L]
    G = nd_ext[:, idx]                                     # (NPC, k_eff, S)
    ndv = G.transpose(1, 0, 2).reshape(k_eff, R)
    ndvT = np.ascontiguousarray(
        ndv.reshape(k_eff, 2, 128).transpose(2, 0, 1)).astype(np.float32)

    return dict(giT=giT, h0T=h0T, tgT=tgT, negsT=negsT, ndvT=ndvT)


def _prep_weights(inputs):
    W_hh = np.asarray(inputs["W_hh"], np.float32)
    b_hh = np.asarray(inputs["b_hh"], np.float32)
    W1 = np.asarray(inputs["W1"], np.float32)
    b1 = np.asarray(inputs["b1"], np.float32)
    W2 = np.asarray(inputs["W2"], np.float32)
    b2 = np.asarray(inputs["b2"], np.float32)
    W3 = np.asarray(inputs["W3"], np.float32)
    b3 = np.asarray(inputs["b3"], np.float32)

    d = {}
    d["w_hh8"] = np.ascontiguousarray(
        W_hh.T.reshape(2, 2, 128, 1536).transpose(0, 2, 1, 3)).astype(F8)
    def pack8(WT):
        # [t, ki, ko, m] with contract index = t*256 + ko*128 + ki
        return np.ascontiguousarray(
            WT.reshape(2, 2, 128, WT.shape[1]).transpose(0, 2, 1, 3)).astype(F8)
    d["w1a8"] = pack8(W1[:, :512].T.copy())
    d["w1b8"] = pack8(W1[:, 512:].T.copy())
    d["w28"] = pack8(W2.T.copy())
    d["w3T"] = np.ascontiguousarray(W3[0].reshape(4, 128).T).astype(BF16)
    bg = np.zeros((128, 16), np.float32)
    for cc in range(12):
        bg[:, cc] = b_hh[cc * 128:(cc + 1) * 128]
    d["bgates"] = bg
    d["b1T"] = np.ascontiguousarray(b1.reshape(4, 128).T).astype(np.float32)
    d["b2T"] = np.ascontiguousarray(b2.reshape(4, 128).T).astype(np.float32)
    d["idt"] = np.eye(128, dtype=BF16)
    d["b3f"] = float(b3.reshape(-1)[0])
    return d


# ------------------------------------------------------------- device program

def _build_program(u_list, k_eff, b3f):
    nc = bacc.Bacc("TRN2", target_bir_lowering=False, debug=False, num_devices=NC)

    di = {}
    def inp(name, shape, dt):
        di[name] = nc.dram_tensor(name, list(shape), dt, kind="ExternalInput")
        return di[name]

    d_whh = inp("w_hh8", (2, 128, 2, 1536), DT.float8e4)
    d_w1a = inp("w1a8", (2, 128, 2, 512), DT.float8e4)
    d_w1b = inp("w1b8", (2, 128, 2, 512), DT.float8e4)
    d_w2 = inp("w28", (2, 128, 2, 512), DT.float8e4)
    d_w3 = inp("w3T", (128, 4), DT.bfloat16)
    d_bg = inp("bgates", (128, 16), DT.float32)
    d_b1 = inp("b1T", (128, 4), DT.float32)
    d_b2 = inp("b2T", (128, 4), DT.float32)
    d_idt = inp("idt", (128, 128), DT.bfloat16)
    d_gi = inp("giT", (k_eff, 128, 12, R), DT.bfloat16)
    d_h0 = inp("h0T", (4, 128, R), DT.float8e4)
    d_tg = inp("tgT", (4, 128, BR), DT.float8e4)
    d_negs = inp("negsT", (4, 128, NEG * BR), DT.float8e4)
    d_ndv = inp("ndvT", (128, k_eff, 2), DT.float32)
    d_out = nc.dram_tensor("out", [1, 4], DT.float32, kind="ExternalOutput")

    f32 = DT.float32
    bf16 = DT.bfloat16
    f8 = DT.float8e4

    with tile.TileContext(nc) as tc:
        with (
            tc.tile_pool(name="const", bufs=1) as cp,
            tc.tile_pool(name="gruw", bufs=2) as gp,
            tc.tile_pool(name="mlpw", bufs=3) as mp,
            tc.tile_pool(name="psg", bufs=1, space="PSUM") as pg,
            tc.tile_pool(name="psm", bufs=2, space="PSUM") as pm,
        ):
            # PSUM budget (8 banks): pg holds the GRU r-gate and g-gate
            # tiles (2+2 banks); pm ([128,2,512] x 2 bufs = 4 banks) holds
            # the GRU z-gate psums during the recurrence and the MLP
            # L1/L2/L3 psums afterwards.
            # ------------------------------------------------ constant loads
            whh = cp.tile([128, 2, 2, 1536], f8, tag="whh")
            for th in range(2):
                nc.sync.dma_start(out=whh[:, th, :, :], in_=d_whh[th])
            w1a = cp.tile([128, 2, 2, 512], f8, tag="w1a")
            w1b = cp.tile([128, 2, 2, 512], f8, tag="w1b")
            w2 = cp.tile([128, 2, 2, 512], f8, tag="w2")
            for (t, d) in ((w1a, d_w1a), (w1b, d_w1b), (w2, d_w2)):
                for th in range(2):
                    nc.sync.dma_start(out=t[:, th, :, :], in_=d[th])
            w3 = cp.tile([128, 4], bf16, tag="w3")
            nc.sync.dma_start(out=w3[:], in_=d_w3[:])
            bg = cp.tile([128, 16], f32, tag="bg")
            nc.sync.dma_start(out=bg[:], in_=d_bg[:])
            b1 = cp.tile([128, 4], f32, tag="b1")
            nc.sync.dma_start(out=b1[:], in_=d_b1[:])
            b2 = cp.tile([128, 4], f32, tag="b2")
            nc.sync.dma_start(out=b2[:], in_=d_b2[:])
            idt = cp.tile([128, 128], bf16, tag="idt")
            nc.sync.dma_start(out=idt[:], in_=d_idt[:])
            tg = cp.tile([128, 4, BR], f8, tag="tg")
            for kc in range(4):
                nc.sync.dma_start(out=tg[:, kc, :], in_=d_tg[kc])
            ndv = cp.tile([128, k_eff, 2], f32, tag="ndv")
            nc.sync.dma_start(out=ndv[:], in_=d_ndv[:])

            # ------------------------------------------------ forward mask
            prod = cp.tile([128, k_eff, 2], f32, tag="prod")
            nc.vector.tensor_scalar(prod[:, 0, :], ndv[:, 0, :], 0.0, None,
                                    op0=ALU.is_gt)
            for k in range(1, k_eff):
                nc.vector.scalar_tensor_tensor(
                    prod[:, k, :], in0=ndv[:, k, :], scalar=0.0,
                    in1=prod[:, k - 1, :], op0=ALU.is_gt, op1=ALU.mult)
            mfT = cp.tile([128, 2 * F], f32, tag="mfT")
            for fi, u in enumerate(u_list):
                nc.vector.tensor_copy(mfT[:, 2 * fi:2 * fi + 2], prod[:, u, :])

            # ------------------------------------------------ GRU
            # r gates in a pg tile, z gates in a pm tile (idle during the
            # recurrence), g gates in a second pg tile.  gi(r/z) (with
            # b_ih + b_hh baked in on host) is accumulated by
            # identity-stationary matmuls so r/z evict as pure
            # scalar-engine sigmoids.  Whh matmuls run th-outer so the
            # next step can start as soon as the first half of h8 lands.
            # The elementwise tail is split in halves for the same
            # reason; e = z*h runs on GpSimd (off the critical path).
            DRM = mybir.MatmulPerfMode.DoubleRow
            h8_prev = gp.tile([128, 4, R], f8, tag="h8")
            for kc in range(4):
                nc.sync.dma_start(out=h8_prev[:, kc, :], in_=d_h0[kc])
            predsT = cp.tile([128, 4, BR], f8, tag="preds")

            for k in range(k_eff):
                gi = gp.tile([128, 12, R], bf16, tag="gi", bufs=3)
                nc.sync.dma_start(out=gi[:], in_=d_gi[k])
                psr = pg.tile([128, 4, R], f32, tag="pr")
                psz = pm.tile([128, 2, 512], f32, tag="pm")
                psg = pg.tile([128, 4, R], f32, tag="pgg")
                nc.tensor.matmul(psr[:, 0:2, :], idt[:], gi[:, 0:2, :],
                                 start=True, stop=False)
                nc.tensor.matmul(psr[:, 2:4, :], idt[:], gi[:, 2:4, :],
                                 start=True, stop=False)
                nc.tensor.matmul(psz[:, 0, :], idt[:], gi[:, 4:6, :],
                                 start=True, stop=False)
                nc.tensor.matmul(psz[:, 1, :], idt[:], gi[:, 6:8, :],
                                 start=True, stop=False)
                for th in range(2):
                    mv = h8_prev[:, 2 * th:2 * th + 2, :]
                    for gc in range(4):
                        nc.tensor.matmul(
                            psr[:, gc, :],
                            whh[:, th, :, gc * 128:(gc + 1) * 128], mv,
                            start=False, stop=(th == 1), perf_mode=DRM)
                    for j in range(4):
                        gc = 4 + j
                        nc.tensor.matmul(
                            psz[:, j // 2, (j % 2) * R:(j % 2 + 1) * R],
                            whh[:, th, :, gc * 128:(gc + 1) * 128], mv,
                            start=False, stop=(th == 1), perf_mode=DRM)
                    for c in range(4):
                        gc = 8 + c
                        nc.tensor.matmul(
                            psg[:, c, :],
                            whh[:, th, :, gc * 128:(gc + 1) * 128], mv,
                            start=(th == 0), stop=(th == 1), perf_mode=DRM)
                r_sb = gp.tile([128, 4, R], bf16, tag="r")
                z_sb = gp.tile([128, 4, R], bf16, tag="z")
                w1m = gp.tile([128, 4, R], bf16, tag="w1m")
                e_sb = gp.tile([128, 4, R], bf16, tag="e")
                nc.scalar.activation(r_sb[:], psr[:], AF.Sigmoid)
                nc.scalar.activation(z_sb[:], psz[:], AF.Sigmoid)
                nc.gpsimd.tensor_mul(e_sb[:, 0:2, :], z_sb[:, 0:2, :],
                                     h8_prev[:, 0:2, :])
                nc.gpsimd.tensor_mul(e_sb[:, 2:4, :], z_sb[:, 2:4, :],
                                     h8_prev[:, 2:4, :])
                t_sb = gp.tile([128, 4, R], bf16, tag="t", bufs=1)
                u_sb = gp.tile([128, 4, R], bf16, tag="u", bufs=1)
                g_sb = gp.tile([128, 4, R], bf16, tag="g")
                gw = gp.tile([128, 4, R], bf16, tag="gw", bufs=1)
                h8_new = gp.tile([128, 4, R], f8, tag="h8")
                for c in range(2):
                    nc.vector.scalar_tensor_tensor(
                        t_sb[:, c, :], in0=psg[:, c, :],
                        scalar=bg[:, 8 + c:9 + c],
                        in1=r_sb[:, c, :], op0=ALU.add, op1=ALU.mult)
                nc.vector.tensor_add(u_sb[:, 0:2, :], gi[:, 8:10, :],
                                     t_sb[:, 0:2, :])
                nc.scalar.activation(g_sb[:, 0:2, :], u_sb[:, 0:2, :],
                                     AF.Tanh)
                nc.vector.tensor_scalar(w1m[:], z_sb[:], -1.0, 1.0,
                                        op0=ALU.mult, op1=ALU.add)
                nc.vector.tensor_mul(gw[:, 0:2, :], g_sb[:, 0:2, :],
                                     w1m[:, 0:2, :])
                nc.vector.tensor_add(h8_new[:, 0:2, :], gw[:, 0:2, :],
                                     e_sb[:, 0:2, :])
                for c in range(2, 4):
                    nc.vector.scalar_tensor_tensor(
                        t_sb[:, c, :], in0=psg[:, c, :],
                        scalar=bg[:, 8 + c:9 + c],
                        in1=r_sb[:, c, :], op0=ALU.add, op1=ALU.mult)
                nc.vector.tensor_add(u_sb[:, 2:4, :], gi[:, 10:12, :],
                                     t_sb[:, 2:4, :])
                nc.scalar.activation(g_sb[:, 2:4, :], u_sb[:, 2:4, :],
                                     AF.Tanh)
                nc.vector.tensor_mul(gw[:, 2:4, :], g_sb[:, 2:4, :],
                                     w1m[:, 2:4, :])
                nc.vector.tensor_add(h8_new[:, 2:4, :], gw[:, 2:4, :],
                                     e_sb[:, 2:4, :])
                h8_prev = h8_new
                for fi, u in enumerate(u_list):
                    if u == k:
                        nc.gpsimd.tensor_copy(
                            predsT[:, :, fi * R:(fi + 1) * R], h8_new[:])

            # ------------------------------------- preds @ W1a + b1 (once)
            p1a = cp.tile([128, 4, BR], bf16, tag="p1a")
            for cc in range(4):
                psp = pm.tile([128, 2, 512], f32, tag="pm")
                for rt in range(2):
                    sl = slice(rt * 512, (rt + 1) * 512)
                    for th in range(2):
                        nc.tensor.matmul(
                            psp[:, rt, :],
                            w1a[:, th, :, cc * 128:(cc + 1) * 128],
                            predsT[:, 2 * th:2 * th + 2, sl],
                            start=(th == 0), stop=(th == 1), perf_mode=DRM)
                nc.scalar.activation(p1a[:, cc, :], psp[:], AF.Identity,
                                     bias=b1[:, cc:cc + 1])

            # ------------------------------------------------ blocks
            # Software-pipelined: L1 of block b+1 is issued before L2/L3
            # of block b so the tensor engine always has independent
            # matmuls while block b's y1 evictions (vector add of the
            # shared preds term + relu-cast, split across scalar/vector)
            # drain.
            logits = cp.tile([128, NBLK, 8], f32, tag="logits")

            def issue_l1(b):
                if b < NEG:
                    xt = mp.tile([128, 4, BR], f8, tag="negsx")
                    for kc in range(4):
                        nc.sync.dma_start(
                            out=xt[:, kc, :],
                            in_=d_negs[kc][:, b * BR:(b + 1) * BR])
                else:
                    xt = tg
                y1 = mp.tile([128, 4, BR], f8, tag="y1", bufs=2)
                y1t = mp.tile([128, 4, BR], bf16, tag="y1t", bufs=2)
                for cc in range(4):
                    psb = pm.tile([128, 2, 512], f32, tag="pm")
                    for rt in range(2):
                        sl = slice(rt * 512, (rt + 1) * 512)
                        for th in range(2):
                            nc.tensor.matmul(
                                psb[:, rt, :],
                                w1b[:, th, :, cc * 128:(cc + 1) * 128],
                                xt[:, 2 * th:2 * th + 2, sl],
                                start=(th == 0), stop=(th == 1), perf_mode=DRM)
                    nc.vector.tensor_add(y1t[:, cc, :], psb[:], p1a[:, cc, :])
                    if cc < 2:
                        nc.scalar.activation(y1[:, cc, :], y1t[:, cc, :],
                                             AF.Relu)
                    else:
                        nc.vector.tensor_scalar(y1[:, cc, :], y1t[:, cc, :],
                                                0.0, None, op0=ALU.max)
                return y1

            def issue_l23(b, y1):
                y2 = mp.tile([128, 4, BR], bf16, tag="y2", bufs=2)
                for cc in range(4):
                    psb = pm.tile([128, 2, 512], f32, tag="pm")
                    for rt in range(2):
                        sl = slice(rt * 512, (rt + 1) * 512)
                        for th in range(2):
                            nc.tensor.matmul(
                                psb[:, rt, :],
                                w2[:, th, :, cc * 128:(cc + 1) * 128],
                                y1[:, 2 * th:2 * th + 2, sl],
                                start=(th == 0), stop=(th == 1), perf_mode=DRM)
                    nc.scalar.activation(y2[:, cc, :], psb[:], AF.Relu,
                                         bias=b2[:, cc:cc + 1])
                ps3 = pm.tile([128, 2, 512], f32, tag="pm")
                for col in range(8):
                    for kc in range(4):
                        nc.tensor.matmul(
                            ps3[:, 0, col:col + 1],
                            y2[:, kc, col * 128:(col + 1) * 128],
                            w3[:, kc:kc + 1], start=(kc == 0), stop=(kc == 3))
                nc.scalar.activation(logits[:, b, :], ps3[:, 0, 0:8], AF.Copy)

            pend = None
            for b in range(NBLK):
                y1b = issue_l1(b)
                if pend is not None:
                    issue_l23(*pend)
                pend = (b, y1b)
            issue_l23(*pend)

            # ------------------------------------- softplus + sums
            # softplus(t) = relu(t) - ln(sigmoid(|t|)); whole-tensor ACT ops
            # keep the activation-table sequence to a single switch.
            partials = cp.tile([128, NBLK + 1], f32, tag="partials")
            sp_a = cp.tile([128, NBLK, 8], f32, tag="sp_a")
            sp_l = cp.tile([128, NBLK, 8], f32, tag="sp_l")
            sp_r = cp.tile([128, NBLK, 8], f32, tag="sp_r")
            sp_d = cp.tile([128, 8], f32, tag="sp_d")
            nc.scalar.activation(sp_a[:], logits[:], AF.Abs, bias=b3f)
            nc.scalar.activation(sp_a[:], sp_a[:], AF.Sigmoid)
            nc.scalar.activation(sp_l[:], sp_a[:], AF.Ln)
            nc.scalar.activation(sp_r[:, :NEG, :], logits[:, :NEG, :],
                                 AF.Relu, bias=b3f)
            nc.scalar.activation(sp_r[:, NEG, :], logits[:, NEG, :],
                                 AF.Relu, bias=-b3f, scale=-1.0)
            nc.vector.tensor_sub(sp_r[:], sp_r[:], sp_l[:])
            for b in range(NBLK):
                nc.vector.tensor_mul(sp_d[:], sp_r[:, b, :], mfT[:])
                nc.vector.tensor_reduce(partials[:, b:b + 1], sp_d[:],
                                        mybir.AxisListType.X, ALU.add)
            nc.vector.tensor_reduce(partials[:, NBLK:NBLK + 1], mfT[:],
                                    mybir.AxisListType.X, ALU.add)

            vcol = cp.tile([128, 4], f32, tag="vcol")
            nc.vector.tensor_copy(vcol[:, 0:1], partials[:, NEG:NEG + 1])
            nc.vector.tensor_reduce(vcol[:, 1:2], partials[:, 0:NEG],
                                    mybir.AxisListType.X, ALU.add)
            nc.vector.tensor_copy(vcol[:, 2:3], partials[:, NBLK:NBLK + 1])
            nc.any.memset(vcol[:, 3:4], 0.0)
            ones = cp.tile([128, 1], f32, tag="ones")
            nc.any.memset(ones[:], 1.0)
            psf = pm.tile([128, 2, 512], f32, tag="pm")
            nc.tensor.matmul(psf[0:1, 0, 0:4], ones[:], vcol[:],
                             start=True, stop=True)
            out_sb = cp.tile([1, 4], f32, tag="out_sb")
            nc.scalar.activation(out_sb[:], psf[0:1, 0, 0:4], AF.Copy)
            nc.sync.dma_start(out=d_out[:], in_=out_sb[:])

    nc.finalize()
    return nc


def _get_program(u_list, k_eff, b3f):
    key = (tuple(u_list), k_eff, float(b3f))
    if key not in _PROGRAM_CACHE:
        _PROGRAM_CACHE[key] = _build_program(u_list, k_eff, b3f)
    return _PROGRAM_CACHE[key]


# ------------------------------------------------------------------ kernel

def kernel(**inputs):
    u_list = [int(x) for x in np.asarray(inputs["unroll_subsample"]).reshape(-1)]
    k_eff = max(u_list) + 1
    w = _prep_weights(inputs)
    nc = _get_program(u_list, k_eff, w["b3f"])

    wmaps = {k: v for k, v in w.items() if k != "b3f"}
    in_maps = []
    for c in range(NC):
        m = dict(wmaps)
        m.update(_prep_core(c, inputs, u_list, k_eff))
        in_maps.append(m)

    res = bass_utils.run_bass_kernel_spmd(nc, in_maps, list(range(NC)))
    P = Ng = D = 0.0
    for c in range(NC):
        o = np.asarray(res.results[c]["out"], np.float64)
        P += o[0, 0]
        Ng += o[0, 1]
        D += o[0, 2]
    loss = COEFF * (P / D + Ng / (D * NEG))
    return np.float32(loss)


# revision 9
# speedup vs baseline: 1.3023x; 1.0317x over previous
"""Trainium2 Bass kernel for the CPCA auxiliary loss (nn_CPCA_51754355917033).

Strategy (data-parallel over the env/batch dim n, 16 envs per core):
  - Host side (sharding prep): every gather baked into per-core contiguous
    device inputs -- action-embedding lookup folded through W_ih (with the
    r/z half of b_hh pre-added), h0 gather (fp8), target gather, negative
    gather, not_dones gather.  All matmul operands pre-transposed
    (contraction dim on partitions).
  - Device GRU: r/z input-gate terms are accumulated into PSUM with
    identity-stationary matmuls, so the r/z eviction is a single
    scalar-engine sigmoid straight out of PSUM (1-z uses sigmoid(-x) via
    scale=-1).  Hidden state is kept in fp8 only.
  - Device MLP: preds @ W1a + b1 is computed ONCE (not per block); each of
    the 21 blocks only runs the negs/tg half of L1 and injects the shared
    term at eviction time (vector add + scalar relu-cast).
  - Host combines the 8 cores' (pos_sum, neg_sum, denom) partials.
"""

import numpy as np
import ml_dtypes

import concourse.bass as bass
import concourse.mybir as mybir
import concourse.tile as tile
from concourse import bacc
from concourse import bass_utils

BF16 = ml_dtypes.bfloat16
F8 = ml_dtypes.float8_e4m3
DT = mybir.dt
AF = mybir.ActivationFunctionType
ALU = mybir.AluOpType

N, T, H, K, S, F, EMB, NLOG, NEG = 128, 512, 512, 16, 16, 4, 32, 18, 20
COEFF = 0.1
NC = 8
NPC = N // NC          # envs per core
R = NPC * S            # GRU rows per core (256)
L = T - 1
NBLK = NEG + 1         # 20 negative g-blocks + 1 positive block
BR = F * R             # rows per block (1024)

_PROGRAM_CACHE = {}


# ----------------------------------------------------------------- host prep

def _prep_core(c, inputs, u_list, k_eff):
    acts = np.asarray(inputs["actions"])[..., 0]
    nd = np.asarray(inputs["not_dones"])[..., 0]
    ri = np.asarray(inputs["rnn_inputs"], np.float32)
    ro = np.asarray(inputs["rnn_outputs"], np.float32)
    ti = np.asarray(inputs["time_subsample"]).astype(np.int64)
    neg_idx = np.asarray(inputs["neg_idx"]).astype(np.int64)
    emb_tab = np.asarray(inputs["action_embed"], np.float32)

    ns = slice(c * NPC, (c + 1) * NPC)
    idx = np.arange(k_eff)[:, None] + ti[None, :]          # (k_eff, S)

    # gi = emb @ W_ih.T + b_ih folded on host, with the r/z half of b_hh
    # pre-added (those gate pre-activations go straight into PSUM).
    W_ih = np.asarray(inputs["W_ih"], np.float32)
    b_ih = np.asarray(inputs["b_ih"], np.float32)
    b_hh = np.asarray(inputs["b_hh"], np.float32)
    bias = b_ih.copy()
    bias[:2 * H] += b_hh[:2 * H]
    GIE = np.zeros((NLOG + 1, 1536), np.float32)
    GIE[:NLOG] = emb_tab @ W_ih.T + bias
    GIE[NLOG] = bias
    act_ext = np.full((NPC, L + K), NLOG, np.int64)
    act_ext[:, :L] = acts[ns, :L]
    AI = act_ext[:, idx]                                   # (NPC, k_eff, S)
    gi_all = GIE[AI.transpose(1, 0, 2).reshape(k_eff, R)]  # (k_eff, R, 1536)
    giT = np.ascontiguousarray(
        gi_all.transpose(0, 2, 1).reshape(k_eff, 12, 128, R)
        .transpose(0, 2, 1, 3)).astype(BF16)               # (k_eff,128,12,R)

    H0 = ro[ns][:, ti]                                     # (NPC, S, H)
    h0T = np.ascontiguousarray(
        H0.transpose(2, 0, 1).reshape(4, 128, R)).astype(F8)

    ri_ext = np.zeros((NPC, L + K, H), np.float32)
    ri_ext[:, :L] = ri[ns, 1:]
    idx2 = np.asarray(u_list)[:, None] + ti[None, :]       # (F, S)
    TG = ri_ext[:, idx2]                                   # (NPC, F, S, H)
    tgT = np.ascontiguousarray(
        TG.transpose(3, 1, 0, 2).reshape(H, BR).reshape(4, 128, BR)).astype(F8)

    ni = neg_idx.reshape(F, N, S, NEG)[:, ns]              # (F, NPC, S, NEG)
    P = ni.transpose(3, 0, 1, 2).reshape(-1)               # cols in (g, f, j) order
    negs = ri.reshape(N * T, H)[P]
    negsT = np.ascontiguousarray(negs.T.reshape(4, 128, NEG * BR)).astype(F8)

    nd_ext = np.zeros((NPC, L + K), np.float32)
    nd_ext[:, :L] = nd[ns, :L]
    G = nd_ext[:, idx]                                     # (NPC, k_eff, S)
    ndv = G.transpose(1, 0, 2).reshape(k_eff, R)
    ndvT = np.ascontiguousarray(
        ndv.reshape(k_eff, 2, 128).transpose(2, 0, 1)).astype(np.float32)

    return dict(giT=giT, h0T=h0T, tgT=tgT, negsT=negsT, ndvT=ndvT)


def _prep_weights(inputs):
    W_hh = np.asarray(inputs["W_hh"], np.float32)
    b_hh = np.asarray(inputs["b_hh"], np.float32)
    W1 = np.asarray(inputs["W1"], np.float32)
    b1 = np.asarray(inputs["b1"], np.float32)
    W2 = np.asarray(inputs["W2"], np.float32)
    b2 = np.asarray(inputs["b2"], np.float32)
    W3 = np.asarray(inputs["W3"], np.float32)
    b3 = np.asarray(inputs["b3"], np.float32)

    d = {}
    d["w_hh8"] = np.ascontiguousarray(
        W_hh.T.reshape(2, 2, 128, 1536).transpose(0, 2, 1, 3)).astype(F8)
    def pack8(WT):
        # [t, ki, ko, m] with contract index = t*256 + ko*128 + ki
        return np.ascontiguousarray(
            WT.reshape(2, 2, 128, WT.shape[1]).transpose(0, 2, 1, 3)).astype(F8)
    d["w1a8"] = pack8(W1[:, :512].T.copy())
    d["w1b8"] = pack8(W1[:, 512:].T.copy())
    d["w28"] = pack8(W2.T.copy())
    d["w3T"] = np.ascontiguousarray(W3[0].reshape(4, 128).T).astype(BF16)
    bg = np.zeros((128, 16), np.float32)
    for cc in range(12):
        bg[:, cc] = b_hh[cc * 128:(cc + 1) * 128]
    d["bgates"] = bg
    d["b1T"] = np.ascontiguousarray(b1.reshape(4, 128).T).astype(np.float32)
    d["b2T"] = np.ascontiguousarray(b2.reshape(4, 128).T).astype(np.float32)
    d["idt"] = np.eye(128, dtype=BF16)
    d["b3f"] = float(b3.reshape(-1)[0])
    return d


# ------------------------------------------------------------- device program

def _build_program(u_list, k_eff, b3f):
    nc = bacc.Bacc("TRN2", target_bir_lowering=False, debug=False, num_devices=NC)

    di = {}
    def inp(name, shape, dt):
        di[name] = nc.dram_tensor(name, list(shape), dt, kind="ExternalInput")
        return di[name]

    d_whh = inp("w_hh8", (2, 128, 2, 1536), DT.float8e4)
    d_w1a = inp("w1a8", (2, 128, 2, 512), DT.float8e4)
    d_w1b = inp("w1b8", (2, 128, 2, 512), DT.float8e4)
    d_w2 = inp("w28", (2, 128, 2, 512), DT.float8e4)
    d_w3 = inp("w3T", (128, 4), DT.bfloat16)
    d_bg = inp("bgates", (128, 16), DT.float32)
    d_b1 = inp("b1T", (128, 4), DT.float32)
    d_b2 = inp("b2T", (128, 4), DT.float32)
    d_idt = inp("idt", (128, 128), DT.bfloat16)
    d_gi = inp("giT", (k_eff, 128, 12, R), DT.bfloat16)
    d_h0 = inp("h0T", (4, 128, R), DT.float8e4)
    d_tg = inp("tgT", (4, 128, BR), DT.float8e4)
    d_negs = inp("negsT", (4, 128, NEG * BR), DT.float8e4)
    d_ndv = inp("ndvT", (128, k_eff, 2), DT.float32)
    d_out = nc.dram_tensor("out", [1, 4], DT.float32, kind="ExternalOutput")

    f32 = DT.float32
    bf16 = DT.bfloat16
    f8 = DT.float8e4

    with tile.TileContext(nc) as tc:
        with (
            tc.tile_pool(name="const", bufs=1) as cp,
            tc.tile_pool(name="gruw", bufs=2) as gp,
            tc.tile_pool(name="mlpw", bufs=3) as mp,
            tc.tile_pool(name="psg", bufs=1, space="PSUM") as pg,
            tc.tile_pool(name="psm", bufs=2, space="PSUM") as pm,
        ):
            # PSUM budget (8 banks): pg holds the GRU r-gate and g-gate
            # tiles (2+2 banks); pm ([128,2,512] x 2 bufs = 4 banks) holds
            # the GRU z-gate psums during the recurrence and the MLP
            # L1/L2/L3 psums afterwards.
            # ------------------------------------------------ constant loads
            whh = cp.tile([128, 2, 2, 1536], f8, tag="whh")
            for th in range(2):
                nc.sync.dma_start(out=whh[:, th, :, :], in_=d_whh[th])
            w1a = cp.tile([128, 2, 2, 512], f8, tag="w1a")
            w1b = cp.tile([128, 2, 2, 512], f8, tag="w1b")
            w2 = cp.tile([128, 2, 2, 512], f8, tag="w2")
            for (t, d) in ((w1a, d_w1a), (w1b, d_w1b), (w2, d_w2)):
                for th in range(2):
                    nc.sync.dma_start(out=t[:, th, :, :], in_=d[th])
            w3 = cp.tile([128, 4], bf16, tag="w3")
            nc.sync.dma_start(out=w3[:], in_=d_w3[:])
            bg = cp.tile([128, 16], f32, tag="bg")
            nc.sync.dma_start(out=bg[:], in_=d_bg[:])
            b1 = cp.tile([128, 4], f32, tag="b1")
            nc.sync.dma_start(out=b1[:], in_=d_b1[:])
            b2 = cp.tile([128, 4], f32, tag="b2")
            nc.sync.dma_start(out=b2[:], in_=d_b2[:])
            idt = cp.tile([128, 128], bf16, tag="idt")
            nc.sync.dma_start(out=idt[:], in_=d_idt[:])
            tg = cp.tile([128, 4, BR], f8, tag="tg")
            for kc in range(4):
                nc.sync.dma_start(out=tg[:, kc, :], in_=d_tg[kc])
            ndv = cp.tile([128, k_eff, 2], f32, tag="ndv")
            nc.sync.dma_start(out=ndv[:], in_=d_ndv[:])

            # ------------------------------------------------ forward mask
            prod = cp.tile([128, k_eff, 2], f32, tag="prod")
            nc.vector.tensor_scalar(prod[:, 0, :], ndv[:, 0, :], 0.0, None,
                                    op0=ALU.is_gt)
            for k in range(1, k_eff):
                nc.vector.scalar_tensor_tensor(
                    prod[:, k, :], in0=ndv[:, k, :], scalar=0.0,
                    in1=prod[:, k - 1, :], op0=ALU.is_gt, op1=ALU.mult)
            mfT = cp.tile([128, 2 * F], f32, tag="mfT")
            for fi, u in enumerate(u_list):
                nc.vector.tensor_copy(mfT[:, 2 * fi:2 * fi + 2], prod[:, u, :])

            # ------------------------------------------------ GRU
            # r gates in a pg tile, z gates in a pm tile (idle during the
            # recurrence), g gates in a second pg tile.  gi(r/z) (with
            # b_ih + b_hh baked in on host) is accumulated by
            # identity-stationary matmuls so r/z evict as pure
            # scalar-engine sigmoids.  Whh matmuls run th-outer so the
            # next step can start as soon as the first half of h8 lands.
            # The elementwise tail is split in halves for the same
            # reason; e = z*h runs on GpSimd (off the critical path).
            DRM = mybir.MatmulPerfMode.DoubleRow
            h8_prev = gp.tile([128, 4, R], f8, tag="h8")
            for kc in range(4):
                nc.sync.dma_start(out=h8_prev[:, kc, :], in_=d_h0[kc])
            predsT = cp.tile([128, 4, BR], f8, tag="preds")

            for k in range(k_eff):
                gi = gp.tile([128, 12, R], bf16, tag="gi", bufs=3)
                nc.sync.dma_start(out=gi[:], in_=d_gi[k])
                psr = pg.tile([128, 4, R], f32, tag="pr")
                psz = pm.tile([128, 2, 512], f32, tag="pm")
                psg = pg.tile([128, 4, R], f32, tag="pgg")
                nc.tensor.matmul(psr[:, 0:2, :], idt[:], gi[:, 0:2, :],
                                 start=True, stop=False)
                nc.tensor.matmul(psr[:, 2:4, :], idt[:], gi[:, 2:4, :],
                                 start=True, stop=False)
                nc.tensor.matmul(psz[:, 0, :], idt[:], gi[:, 4:6, :],
                                 start=True, stop=False)
                nc.tensor.matmul(psz[:, 1, :], idt[:], gi[:, 6:8, :],
                                 start=True, stop=False)
                for th in range(2):
                    mv = h8_prev[:, 2 * th:2 * th + 2, :]
                    for gc in range(4):
                        nc.tensor.matmul(
                            psr[:, gc, :],
                            whh[:, th, :, gc * 128:(gc + 1) * 128], mv,
                            start=False, stop=(th == 1), perf_mode=DRM)
                    for j in range(4):
                        gc = 4 + j
                        nc.tensor.matmul(
                            psz[:, j // 2, (j % 2) * R:(j % 2 + 1) * R],
                            whh[:, th, :, gc * 128:(gc + 1) * 128], mv,
                            start=False, stop=(th == 1), perf_mode=DRM)
                    for c in range(4):
                        gc = 8 + c
                        nc.tensor.matmul(
                            psg[:, c, :],
                            whh[:, th, :, gc * 128:(gc + 1) * 128], mv,
                            start=(th == 0), stop=(th == 1), perf_mode=DRM)
                # tail: h_new = g - z*(g - h_prev); no (1-z) or z*h
                # intermediates, so the z PSUM frees right after its
                # sigmoid and the tail is 3 short vector ops per half.
                r_sb = gp.tile([128, 4, R], bf16, tag="r")
                z_sb = gp.tile([128, 4, R], bf16, tag="z")
                nc.scalar.activation(r_sb[:], psr[:], AF.Sigmoid)
                nc.scalar.activation(z_sb[:], psz[:], AF.Sigmoid)
                t_sb = gp.tile([128, 4, R], bf16, tag="t", bufs=1)
                u_sb = gp.tile([128, 4, R], bf16, tag="u", bufs=1)
                g_sb = gp.tile([128, 4, R], bf16, tag="g")
                d_sb = gp.tile([128, 4, R], bf16, tag="d", bufs=1)
                m_sb = gp.tile([128, 4, R], bf16, tag="m", bufs=1)
                h8_new = gp.tile([128, 4, R], f8, tag="h8")
                for half in range(2):
                    cs = slice(2 * half, 2 * half + 2)
                    gis = slice(8 + 2 * half, 10 + 2 * half)
                    for c in range(2 * half, 2 * half + 2):
                        nc.vector.scalar_tensor_tensor(
                            t_sb[:, c, :], in0=psg[:, c, :],
                            scalar=bg[:, 8 + c:9 + c],
                            in1=r_sb[:, c, :], op0=ALU.add, op1=ALU.mult)
                    nc.vector.tensor_add(u_sb[:, cs, :], gi[:, gis, :],
                                         t_sb[:, cs, :])
                    nc.scalar.activation(g_sb[:, cs, :], u_sb[:, cs, :],
                                         AF.Tanh)
                    nc.vector.tensor_sub(d_sb[:, cs, :], g_sb[:, cs, :],
                                         h8_prev[:, cs, :])
                    nc.vector.tensor_mul(m_sb[:, cs, :], z_sb[:, cs, :],
                                         d_sb[:, cs, :])
                    nc.vector.tensor_sub(h8_new[:, cs, :], g_sb[:, cs, :],
                                         m_sb[:, cs, :])
                h8_prev = h8_new
                for fi, u in enumerate(u_list):
                    if u == k:
                        nc.gpsimd.tensor_copy(
                            predsT[:, :, fi * R:(fi + 1) * R], h8_new[:])

            # ------------------------------------- preds @ W1a + b1 (once)
            p1a = cp.tile([128, 4, BR], bf16, tag="p1a")
            for cc in range(4):
                psp = pm.tile([128, 2, 512], f32, tag="pm")
                for rt in range(2):
                    sl = slice(rt * 512, (rt + 1) * 512)
                    for th in range(2):
                        nc.tensor.matmul(
                            psp[:, rt, :],
                            w1a[:, th, :, cc * 128:(cc + 1) * 128],
                            predsT[:, 2 * th:2 * th + 2, sl],
                            start=(th == 0), stop=(th == 1), perf_mode=DRM)
                nc.scalar.activation(p1a[:, cc, :], psp[:], AF.Identity,
                                     bias=b1[:, cc:cc + 1])

            # ------------------------------------------------ blocks
            # Software-pipelined: L1 of block b+1 is issued before L2/L3
            # of block b so the tensor engine always has independent
            # matmuls while block b's y1 evictions (vector add of the
            # shared preds term + relu-cast, split across scalar/vector)
            # drain.
            logits = cp.tile([128, NBLK, 8], f32, tag="logits")

            def issue_l1(b):
                if b < NEG:
                    xt = mp.tile([128, 4, BR], f8, tag="negsx")
                    for kc in range(4):
                        nc.sync.dma_start(
                            out=xt[:, kc, :],
                            in_=d_negs[kc][:, b * BR:(b + 1) * BR])
                else:
                    xt = tg
                y1 = mp.tile([128, 4, BR], f8, tag="y1", bufs=2)
                y1t = mp.tile([128, 4, BR], bf16, tag="y1t", bufs=2)
                for cc in range(4):
                    psb = pm.tile([128, 2, 512], f32, tag="pm")
                    for rt in range(2):
                        sl = slice(rt * 512, (rt + 1) * 512)
                        for th in range(2):
                            nc.tensor.matmul(
                                psb[:, rt, :],
                                w1b[:, th, :, cc * 128:(cc + 1) * 128],
                                xt[:, 2 * th:2 * th + 2, sl],
                                start=(th == 0), stop=(th == 1), perf_mode=DRM)
                    nc.vector.tensor_add(y1t[:, cc, :], psb[:], p1a[:, cc, :])
                    nc.vector.tensor_scalar(y1[:, cc, :], y1t[:, cc, :],
                                            0.0, None, op0=ALU.max)
                return y1

            def issue_l2(b, y1):
                y2 = mp.tile([128, 4, BR], bf16, tag="y2", bufs=2)
                for cc in range(4):
                    psb = pm.tile([128, 2, 512], f32, tag="pm")
                    for rt in range(2):
                        sl = slice(rt * 512, (rt + 1) * 512)
                        for th in range(2):
                            nc.tensor.matmul(
                                psb[:, rt, :],
                                w2[:, th, :, cc * 128:(cc + 1) * 128],
                                y1[:, 2 * th:2 * th + 2, sl],
                                start=(th == 0), stop=(th == 1), perf_mode=DRM)
                    nc.scalar.activation(y2[:, cc, :], psb[:], AF.Relu,
                                         bias=b2[:, cc:cc + 1])
                return y2

            def issue_l3(b, y2):
                ps3 = pm.tile([128, 2, 512], f32, tag="pm")
                for col in range(8):
                    for kc in range(4):
                        nc.tensor.matmul(
                            ps3[:, 0, col:col + 1],
                            y2[:, kc, col * 128:(col + 1) * 128],
                            w3[:, kc:kc + 1], start=(kc == 0), stop=(kc == 3))
                nc.scalar.activation(logits[:, b, :], ps3[:, 0, 0:8], AF.Copy)

            # two-deep software pipeline: L1(b) | L2(b-1) | L3(b-2), so
            # the tensor engine never waits on an eviction chain
            y1s, y2s = {}, {}
            for b in range(NBLK + 2):
                if b < NBLK:
                    y1s[b] = issue_l1(b)
                if 1 <= b and b - 1 < NBLK:
                    y2s[b - 1] = issue_l2(b - 1, y1s.pop(b - 1))
                if 2 <= b:
                    issue_l3(b - 2, y2s.pop(b - 2))

            # ------------------------------------- softplus + sums
            # softplus(t) = relu(t) - ln(sigmoid(|t|)); whole-tensor ACT ops
            # keep the activation-table sequence to a single switch.
            partials = cp.tile([128, NBLK + 1], f32, tag="partials")
            sp_a = cp.tile([128, NBLK, 8], f32, tag="sp_a")
            sp_l = cp.tile([128, NBLK, 8], f32, tag="sp_l")
            sp_r = cp.tile([128, NBLK, 8], f32, tag="sp_r")
            sp_d = cp.tile([128, 8], f32, tag="sp_d")
            nc.scalar.activation(sp_a[:], logits[:], AF.Abs, bias=b3f)
            nc.scalar.activation(sp_a[:], sp_a[:], AF.Sigmoid)
            nc.scalar.activation(sp_l[:], sp_a[:], AF.Ln)
            nc.scalar.activation(sp_r[:, :NEG, :], logits[:, :NEG, :],
                                 AF.Relu, bias=b3f)
            nc.scalar.activation(sp_r[:, NEG, :], logits[:, NEG, :],
                                 AF.Relu, bias=-b3f, scale=-1.0)
            nc.vector.tensor_sub(sp_r[:], sp_r[:], sp_l[:])
            for b in range(NBLK):
                nc.vector.tensor_mul(sp_d[:], sp_r[:, b, :], mfT[:])
                nc.vector.tensor_reduce(partials[:, b:b + 1], sp_d[:],
                                        mybir.AxisListType.X, ALU.add)
            nc.vector.tensor_reduce(partials[:, NBLK:NBLK + 1], mfT[:],
                                    mybir.AxisListType.X, ALU.add)

            vcol = cp.tile([128, 4], f32, tag="vcol")
            nc.vector.tensor_copy(vcol[:, 0:1], partials[:, NEG:NEG + 1])
            nc.vector.tensor_reduce(vcol[:, 1:2], partials[:, 0:NEG],
                                    mybir.AxisListType.X, ALU.add)
            nc.vector.tensor_copy(vcol[:, 2:3], partials[:, NBLK:NBLK + 1])
            nc.any.memset(vcol[:, 3:4], 0.0)
            ones = cp.tile([128, 1], f32, tag="ones")
            nc.any.memset(ones[:], 1.0)
            psf = pm.tile([128, 2, 512], f32, tag="pm")
            nc.tensor.matmul(psf[0:1, 0, 0:4], ones[:], vcol[:],
                             start=True, stop=True)
            out_sb = cp.tile([1, 4], f32, tag="out_sb")
            nc.scalar.activation(out_sb[:], psf[0:1, 0, 0:4], AF.Copy)
            nc.sync.dma_start(out=d_out[:], in_=out_sb[:])

    nc.finalize()
    return nc


def _get_program(u_list, k_eff, b3f):
    key = (tuple(u_list), k_eff, float(b3f))
    if key not in _PROGRAM_CACHE:
        _PROGRAM_CACHE[key] = _build_program(u_list, k_eff, b3f)
    return _PROGRAM_CACHE[key]


# ------------------------------------------------------------------ kernel

def kernel(**inputs):
    u_list = [int(x) for x in np.asarray(inputs["unroll_subsample"]).reshape(-1)]
    k_eff = max(u_list) + 1
    w = _prep_weights(inputs)
    nc = _get_program(u_list, k_eff, w["b3f"])

    wmaps = {k: v for k, v in w.items() if k != "b3f"}
    in_maps = []
    for c in range(NC):
        m = dict(wmaps)
        m.update(_prep_core(c, inputs, u_list, k_eff))
        in_maps.append(m)

    res = bass_utils.run_bass_kernel_spmd(nc, in_maps, list(range(NC)))
    P = Ng = D = 0.0
    for c in range(NC):
        o = np.asarray(res.results[c]["out"], np.float64)
        P += o[0, 0]
        Ng += o[0, 1]
        D += o[0, 2]
    loss = COEFF * (P / D + Ng / (D * NEG))
    return np.float32(loss)


# revision 13
# speedup vs baseline: 1.3652x; 1.0482x over previous
"""Trainium2 Bass kernel for the CPCA auxiliary loss (nn_CPCA_51754355917033).

Strategy (data-parallel over the env/batch dim n, 16 envs per core):
  - Host side (sharding prep): every gather baked into per-core contiguous
    device inputs -- action-embedding lookup folded through W_ih (with the
    r/z half of b_hh pre-added), h0 gather (fp8), target gather, negative
    gather, not_dones gather.  All matmul operands pre-transposed
    (contraction dim on partitions).
  - Device GRU: r/z input-gate terms are accumulated into PSUM with
    identity-stationary matmuls, so the r/z eviction is a single
    scalar-engine sigmoid straight out of PSUM (1-z uses sigmoid(-x) via
    scale=-1).  Hidden state is kept in fp8 only.
  - Device MLP: preds @ W1a + b1 is computed ONCE (not per block); each of
    the 21 blocks only runs the negs/tg half of L1 and injects the shared
    term at eviction time (vector add + scalar relu-cast).
  - Host combines the 8 cores' (pos_sum, neg_sum, denom) partials.
"""

import numpy as np
import ml_dtypes

import concourse.bass as bass
import concourse.mybir as mybir
import concourse.tile as tile
from concourse import bacc
from concourse import bass_utils

BF16 = ml_dtypes.bfloat16
F8 = ml_dtypes.float8_e4m3
DT = mybir.dt
AF = mybir.ActivationFunctionType
ALU = mybir.AluOpType

N, T, H, K, S, F, EMB, NLOG, NEG = 128, 512, 512, 16, 16, 4, 32, 18, 20
COEFF = 0.1
NC = 8
NPC = N // NC          # envs per core
R = NPC * S            # GRU rows per core (256)
L = T - 1
NBLK = NEG + 1         # 20 negative g-blocks + 1 positive block
BR = F * R             # rows per block (1024)

_PROGRAM_CACHE = {}


# ----------------------------------------------------------------- host prep

def _prep_core(c, inputs, u_list, k_eff):
    acts = np.asarray(inputs["actions"])[..., 0]
    nd = np.asarray(inputs["not_dones"])[..., 0]
    ri = np.asarray(inputs["rnn_inputs"], np.float32)
    ro = np.asarray(inputs["rnn_outputs"], np.float32)
    ti = np.asarray(inputs["time_subsample"]).astype(np.int64)
    neg_idx = np.asarray(inputs["neg_idx"]).astype(np.int64)
    emb_tab = np.asarray(inputs["action_embed"], np.float32)

    ns = slice(c * NPC, (c + 1) * NPC)
    idx = np.arange(k_eff)[:, None] + ti[None, :]          # (k_eff, S)

    # gi = emb @ W_ih.T + b_ih folded on host, with the r/z half of b_hh
    # pre-added (those gate pre-activations go straight into PSUM).
    W_ih = np.asarray(inputs["W_ih"], np.float32)
    b_ih = np.asarray(inputs["b_ih"], np.float32)
    b_hh = np.asarray(inputs["b_hh"], np.float32)
    bias = b_ih.copy()
    bias[:2 * H] += b_hh[:2 * H]
    GIE = np.zeros((NLOG + 1, 1536), np.float32)
    GIE[:NLOG] = emb_tab @ W_ih.T + bias
    GIE[NLOG] = bias
    act_ext = np.full((NPC, L + K), NLOG, np.int64)
    act_ext[:, :L] = acts[ns, :L]
    AI = act_ext[:, idx]                                   # (NPC, k_eff, S)
    gi_all = GIE[AI.transpose(1, 0, 2).reshape(k_eff, R)]  # (k_eff, R, 1536)
    giT = np.ascontiguousarray(
        gi_all.transpose(0, 2, 1).reshape(k_eff, 12, 128, R)
        .transpose(0, 2, 1, 3)).astype(BF16)               # (k_eff,128,12,R)

    H0 = ro[ns][:, ti]                                     # (NPC, S, H)
    h0T = np.ascontiguousarray(
        H0.transpose(2, 0, 1).reshape(4, 128, R)).astype(F8)

    ri_ext = np.zeros((NPC, L + K, H), np.float32)
    ri_ext[:, :L] = ri[ns, 1:]
    idx2 = np.asarray(u_list)[:, None] + ti[None, :]       # (F, S)
    TG = ri_ext[:, idx2]                                   # (NPC, F, S, H)
    tgT = np.ascontiguousarray(
        TG.transpose(3, 1, 0, 2).reshape(H, BR).reshape(4, 128, BR)).astype(F8)

    ni = neg_idx.reshape(F, N, S, NEG)[:, ns]              # (F, NPC, S, NEG)
    P = ni.transpose(3, 0, 1, 2).reshape(-1)               # cols in (g, f, j) order
    negs = ri.reshape(N * T, H)[P]
    negsT = np.ascontiguousarray(negs.T.reshape(4, 128, NEG * BR)).astype(F8)

    nd_ext = np.zeros((NPC, L + K), np.float32)
    nd_ext[:, :L] = nd[ns, :L]
    G = nd_ext[:, idx]                                     # (NPC, k_eff, S)
    ndv = G.transpose(1, 0, 2).reshape(k_eff, R)
    ndvT = np.ascontiguousarray(
        ndv.reshape(k_eff, 2, 128).transpose(2, 0, 1)).astype(np.float32)

    return dict(giT=giT, h0T=h0T, tgT=tgT, negsT=negsT, ndvT=ndvT)


def _prep_weights(inputs):
    W_hh = np.asarray(inputs["W_hh"], np.float32)
    b_hh = np.asarray(inputs["b_hh"], np.float32)
    W1 = np.asarray(inputs["W1"], np.float32)
    b1 = np.asarray(inputs["b1"], np.float32)
    W2 = np.asarray(inputs["W2"], np.float32)
    b2 = np.asarray(inputs["b2"], np.float32)
    W3 = np.asarray(inputs["W3"], np.float32)
    b3 = np.asarray(inputs["b3"], np.float32)

    d = {}
    d["w_hh8"] = np.ascontiguousarray(
        W_hh.T.reshape(2, 2, 128, 1536).transpose(0, 2, 1, 3)).astype(F8)
    def pack8(WT):
        # [t, ki, ko, m] with contract index = t*256 + ko*128 + ki
        return np.ascontiguousarray(
            WT.reshape(2, 2, 128, WT.shape[1]).transpose(0, 2, 1, 3)).astype(F8)
    d["w1a8"] = pack8(W1[:, :512].T.copy())
    d["w1b8"] = pack8(W1[:, 512:].T.copy())
    d["w28"] = pack8(W2.T.copy())
    d["w3T"] = np.ascontiguousarray(W3[0].reshape(4, 128).T).astype(BF16)
    bg = np.zeros((128, 16), np.float32)
    for cc in range(12):
        bg[:, cc] = b_hh[cc * 128:(cc + 1) * 128]
    d["bgates"] = bg
    d["b1T"] = np.ascontiguousarray(b1.reshape(4, 128).T).astype(np.float32)
    d["b2T"] = np.ascontiguousarray(b2.reshape(4, 128).T).astype(np.float32)
    d["idt"] = np.eye(128, dtype=BF16)
    d["b3f"] = float(b3.reshape(-1)[0])
    return d


# ------------------------------------------------------------- device program

def _build_program(u_list, k_eff, b3f):
    nc = bacc.Bacc("TRN2", target_bir_lowering=False, debug=False, num_devices=NC)

    di = {}
    def inp(name, shape, dt):
        di[name] = nc.dram_tensor(name, list(shape), dt, kind="ExternalInput")
        return di[name]

    d_whh = inp("w_hh8", (2, 128, 2, 1536), DT.float8e4)
    d_w1a = inp("w1a8", (2, 128, 2, 512), DT.float8e4)
    d_w1b = inp("w1b8", (2, 128, 2, 512), DT.float8e4)
    d_w2 = inp("w28", (2, 128, 2, 512), DT.float8e4)
    d_w3 = inp("w3T", (128, 4), DT.bfloat16)
    d_bg = inp("bgates", (128, 16), DT.float32)
    d_b1 = inp("b1T", (128, 4), DT.float32)
    d_b2 = inp("b2T", (128, 4), DT.float32)
    d_idt = inp("idt", (128, 128), DT.bfloat16)
    d_gi = inp("giT", (k_eff, 128, 12, R), DT.bfloat16)
    d_h0 = inp("h0T", (4, 128, R), DT.float8e4)
    d_tg = inp("tgT", (4, 128, BR), DT.float8e4)
    d_negs = inp("negsT", (4, 128, NEG * BR), DT.float8e4)
    d_ndv = inp("ndvT", (128, k_eff, 2), DT.float32)
    d_out = nc.dram_tensor("out", [1, 4], DT.float32, kind="ExternalOutput")

    f32 = DT.float32
    bf16 = DT.bfloat16
    f8 = DT.float8e4

    with tile.TileContext(nc) as tc:
        with (
            tc.tile_pool(name="const", bufs=1) as cp,
            tc.tile_pool(name="gruw", bufs=2) as gp,
            tc.tile_pool(name="mlpw", bufs=3) as mp,
            tc.tile_pool(name="psg", bufs=1, space="PSUM") as pg,
            tc.tile_pool(name="psm", bufs=2, space="PSUM") as pm,
        ):
            # PSUM budget (8 banks): pg holds the GRU r-gate and g-gate
            # tiles (2+2 banks); pm ([128,2,512] x 2 bufs = 4 banks) holds
            # the GRU z-gate psums during the recurrence and the MLP
            # L1/L2/L3 psums afterwards.
            # ------------------------------------------------ constant loads
            # GRU-critical tensors first so step 0 can start ASAP; the
            # MLP weights, targets, and mask inputs load behind them.
            idt = cp.tile([128, 128], bf16, tag="idt")
            nc.sync.dma_start(out=idt[:], in_=d_idt[:])
            bg = cp.tile([128, 16], f32, tag="bg")
            nc.sync.dma_start(out=bg[:], in_=d_bg[:])
            whh = cp.tile([128, 2, 2, 1536], f8, tag="whh")
            for th in range(2):
                nc.sync.dma_start(out=whh[:, th, :, :], in_=d_whh[th])
            w1a = cp.tile([128, 2, 2, 512], f8, tag="w1a")
            w1b = cp.tile([128, 2, 2, 512], f8, tag="w1b")
            w2 = cp.tile([128, 2, 2, 512], f8, tag="w2")
            for (t, d) in ((w1a, d_w1a), (w1b, d_w1b), (w2, d_w2)):
                for th in range(2):
                    nc.sync.dma_start(out=t[:, th, :, :], in_=d[th])
            w3 = cp.tile([128, 4], bf16, tag="w3")
            nc.sync.dma_start(out=w3[:], in_=d_w3[:])
            b1 = cp.tile([128, 4], f32, tag="b1")
            nc.sync.dma_start(out=b1[:], in_=d_b1[:])
            b2 = cp.tile([128, 4], f32, tag="b2")
            nc.sync.dma_start(out=b2[:], in_=d_b2[:])

            # ------------------------------------------------ GRU
            # r gates in a pg tile, z gates in a pm tile (idle during the
            # recurrence), g gates in a second pg tile.  gi(r/z) (with
            # b_ih + b_hh baked in on host) is accumulated by
            # identity-stationary matmuls so r/z evict as pure
            # scalar-engine sigmoids.  Whh matmuls run th-outer so the
            # next step can start as soon as the first half of h8 lands.
            # The elementwise tail is split in halves for the same
            # reason; e = z*h runs on GpSimd (off the critical path).
            DRM = mybir.MatmulPerfMode.DoubleRow
            h8_prev = gp.tile([128, 4, R], f8, tag="h8")
            for kc in range(4):
                nc.sync.dma_start(out=h8_prev[:, kc, :], in_=d_h0[kc])
            predsT = cp.tile([128, 4, BR], f8, tag="preds")

            for k in range(k_eff):
                gi = gp.tile([128, 12, R], bf16, tag="gi", bufs=3)
                nc.sync.dma_start(out=gi[:], in_=d_gi[k])
                psr = pg.tile([128, 4, R], f32, tag="pr")
                psz = pm.tile([128, 2, 512], f32, tag="pm")
                psg = pg.tile([128, 4, R], f32, tag="pgg")
                nc.tensor.matmul(psr[:, 0:2, :], idt[:], gi[:, 0:2, :],
                                 start=True, stop=False)
                nc.tensor.matmul(psr[:, 2:4, :], idt[:], gi[:, 2:4, :],
                                 start=True, stop=False)
                nc.tensor.matmul(psz[:, 0, :], idt[:], gi[:, 4:6, :],
                                 start=True, stop=False)
                nc.tensor.matmul(psz[:, 1, :], idt[:], gi[:, 6:8, :],
                                 start=True, stop=False)
                # gate order r, g, z within each th batch: the r sigmoid
                # and g psums gate the elementwise tail, z is only needed
                # late (at m = z*d)
                for th in range(2):
                    mv = h8_prev[:, 2 * th:2 * th + 2, :]
                    for gc in range(4):
                        nc.tensor.matmul(
                            psr[:, gc, :],
                            whh[:, th, :, gc * 128:(gc + 1) * 128], mv,
                            start=False, stop=(th == 1), perf_mode=DRM)
                    for c in range(4):
                        gc = 8 + c
                        nc.tensor.matmul(
                            psg[:, c, :],
                            whh[:, th, :, gc * 128:(gc + 1) * 128], mv,
                            start=(th == 0), stop=(th == 1), perf_mode=DRM)
                    for j in range(4):
                        gc = 4 + j
                        nc.tensor.matmul(
                            psz[:, j // 2, (j % 2) * R:(j % 2 + 1) * R],
                            whh[:, th, :, gc * 128:(gc + 1) * 128], mv,
                            start=False, stop=(th == 1), perf_mode=DRM)
                # tail: h_new = g - z*(g - h_prev); no (1-z) or z*h
                # intermediates, so the z PSUM frees right after its
                # sigmoid and the tail is 3 short vector ops per half.
                r_sb = gp.tile([128, 4, R], bf16, tag="r")
                z_sb = gp.tile([128, 4, R], bf16, tag="z")
                nc.scalar.activation(r_sb[:], psr[:], AF.Sigmoid)
                nc.scalar.activation(z_sb[:], psz[:], AF.Sigmoid)
                t_sb = gp.tile([128, 4, R], bf16, tag="t", bufs=1)
                u_sb = gp.tile([128, 4, R], bf16, tag="u", bufs=1)
                g_sb = gp.tile([128, 4, R], bf16, tag="g")
                d_sb = gp.tile([128, 4, R], bf16, tag="d", bufs=1)
                m_sb = gp.tile([128, 4, R], bf16, tag="m", bufs=1)
                h8_new = gp.tile([128, 4, R], f8, tag="h8")
                for half in range(2):
                    cs = slice(2 * half, 2 * half + 2)
                    gis = slice(8 + 2 * half, 10 + 2 * half)
                    for c in range(2 * half, 2 * half + 2):
                        nc.vector.scalar_tensor_tensor(
                            t_sb[:, c, :], in0=psg[:, c, :],
                            scalar=bg[:, 8 + c:9 + c],
                            in1=r_sb[:, c, :], op0=ALU.add, op1=ALU.mult)
                    nc.vector.tensor_add(u_sb[:, cs, :], gi[:, gis, :],
                                         t_sb[:, cs, :])
                    nc.scalar.activation(g_sb[:, cs, :], u_sb[:, cs, :],
                                         AF.Tanh)
                    nc.vector.tensor_sub(d_sb[:, cs, :], g_sb[:, cs, :],
                                         h8_prev[:, cs, :])
                    nc.vector.tensor_mul(m_sb[:, cs, :], z_sb[:, cs, :],
                                         d_sb[:, cs, :])
                    nc.vector.tensor_sub(h8_new[:, cs, :], g_sb[:, cs, :],
                                         m_sb[:, cs, :])
                h8_prev = h8_new
                for fi, u in enumerate(u_list):
                    if u == k:
                        nc.gpsimd.tensor_copy(
                            predsT[:, :, fi * R:(fi + 1) * R], h8_new[:])

            # ---------------------------- MLP-only inputs + forward mask
            tg = cp.tile([128, 4, BR], f8, tag="tg")
            for kc in range(4):
                nc.sync.dma_start(out=tg[:, kc, :], in_=d_tg[kc])
            ndv = cp.tile([128, k_eff, 2], f32, tag="ndv")
            nc.sync.dma_start(out=ndv[:], in_=d_ndv[:])
            prod = cp.tile([128, k_eff, 2], f32, tag="prod")
            nc.vector.tensor_scalar(prod[:, 0, :], ndv[:, 0, :], 0.0, None,
                                    op0=ALU.is_gt)
            for k in range(1, k_eff):
                nc.vector.scalar_tensor_tensor(
                    prod[:, k, :], in0=ndv[:, k, :], scalar=0.0,
                    in1=prod[:, k - 1, :], op0=ALU.is_gt, op1=ALU.mult)
            mfT = cp.tile([128, 2 * F], f32, tag="mfT")
            for fi, u in enumerate(u_list):
                nc.vector.tensor_copy(mfT[:, 2 * fi:2 * fi + 2], prod[:, u, :])

            # ------------------------------------- preds @ W1a + b1 (once)
            p1a = cp.tile([128, 4, BR], bf16, tag="p1a")
            for cc in range(4):
                psp = pm.tile([128, 2, 512], f32, tag="pm")
                for rt in range(2):
                    sl = slice(rt * 512, (rt + 1) * 512)
                    for th in range(2):
                        nc.tensor.matmul(
                            psp[:, rt, :],
                            w1a[:, th, :, cc * 128:(cc + 1) * 128],
                            predsT[:, 2 * th:2 * th + 2, sl],
                            start=(th == 0), stop=(th == 1), perf_mode=DRM)
                nc.scalar.activation(p1a[:, cc, :], psp[:], AF.Identity,
                                     bias=b1[:, cc:cc + 1])

            # ------------------------------------------------ blocks
            # Software-pipelined: L1 of block b+1 is issued before L2/L3
            # of block b so the tensor engine always has independent
            # matmuls while block b's y1 evictions (vector add of the
            # shared preds term + relu-cast, split across scalar/vector)
            # drain.
            logits = cp.tile([128, NBLK, 8], f32, tag="logits")

            def issue_l1(b):
                if b < NEG:
                    xt = mp.tile([128, 4, BR], f8, tag="negsx")
                    for kc in range(4):
                        nc.sync.dma_start(
                            out=xt[:, kc, :],
                            in_=d_negs[kc][:, b * BR:(b + 1) * BR])
                else:
                    xt = tg
                y1 = mp.tile([128, 4, BR], f8, tag="y1", bufs=2)
                y1t = mp.tile([128, 4, BR], bf16, tag="y1t", bufs=2)
                for cc in range(4):
                    psb = pm.tile([128, 2, 512], f32, tag="pm")
                    for rt in range(2):
                        sl = slice(rt * 512, (rt + 1) * 512)
                        for th in range(2):
                            nc.tensor.matmul(
                                psb[:, rt, :],
                                w1b[:, th, :, cc * 128:(cc + 1) * 128],
                                xt[:, 2 * th:2 * th + 2, sl],
                                start=(th == 0), stop=(th == 1), perf_mode=DRM)
                    nc.vector.tensor_add(y1t[:, cc, :], psb[:], p1a[:, cc, :])
                    if cc % 2 == 0:
                        nc.scalar.activation(y1[:, cc, :], y1t[:, cc, :],
                                             AF.Relu)
                    else:
                        nc.vector.tensor_scalar(y1[:, cc, :], y1t[:, cc, :],
                                                0.0, None, op0=ALU.max)
                return y1

            def issue_l2(b, y1):
                y2 = mp.tile([128, 4, BR], bf16, tag="y2", bufs=2)
                for cc in range(4):
                    psb = pm.tile([128, 2, 512], f32, tag="pm")
                    for rt in range(2):
                        sl = slice(rt * 512, (rt + 1) * 512)
                        for th in range(2):
                            nc.tensor.matmul(
                                psb[:, rt, :],
                                w2[:, th, :, cc * 128:(cc + 1) * 128],
                                y1[:, 2 * th:2 * th + 2, sl],
                                start=(th == 0), stop=(th == 1), perf_mode=DRM)
                    nc.scalar.activation(y2[:, cc, :], psb[:], AF.Relu,
                                         bias=b2[:, cc:cc + 1])
                return y2

            def issue_l3(b, y2):
                ps3 = pm.tile([128, 2, 512], f32, tag="pm")
                for col in range(8):
                    for kc in range(4):
                        nc.tensor.matmul(
                            ps3[:, 0, col:col + 1],
                            y2[:, kc, col * 128:(col + 1) * 128],
                            w3[:, kc:kc + 1], start=(kc == 0), stop=(kc == 3))
                nc.scalar.activation(logits[:, b, :], ps3[:, 0, 0:8], AF.Copy)

            # two-deep software pipeline: L1(b) | L2(b-1) | L3(b-2), so
            # the tensor engine never waits on an eviction chain
            y1s, y2s = {}, {}
            for b in range(NBLK + 2):
                if b < NBLK:
                    y1s[b] = issue_l1(b)
                if 1 <= b and b - 1 < NBLK:
                    y2s[b - 1] = issue_l2(b - 1, y1s.pop(b - 1))
                if 2 <= b:
                    issue_l3(b - 2, y2s.pop(b - 2))

            # ------------------------------------- softplus + sums
            # softplus(t) = relu(t) - ln(sigmoid(|t|)); whole-tensor ACT ops
            # keep the activation-table sequence to a single switch.
            partials = cp.tile([128, NBLK + 1], f32, tag="partials")
            sp_a = cp.tile([128, NBLK, 8], f32, tag="sp_a")
            sp_l = cp.tile([128, NBLK, 8], f32, tag="sp_l")
            sp_r = cp.tile([128, NBLK, 8], f32, tag="sp_r")
            sp_d = cp.tile([128, 8], f32, tag="sp_d")
            nc.scalar.activation(sp_a[:], logits[:], AF.Abs, bias=b3f)
            nc.scalar.activation(sp_a[:], sp_a[:], AF.Sigmoid)
            nc.scalar.activation(sp_l[:], sp_a[:], AF.Ln)
            nc.scalar.activation(sp_r[:, :NEG, :], logits[:, :NEG, :],
                                 AF.Relu, bias=b3f)
            nc.scalar.activation(sp_r[:, NEG, :], logits[:, NEG, :],
                                 AF.Relu, bias=-b3f, scale=-1.0)
            nc.vector.tensor_sub(sp_r[:], sp_r[:], sp_l[:])
            for b in range(NBLK):
                nc.vector.tensor_mul(sp_d[:], sp_r[:, b, :], mfT[:])
                nc.vector.tensor_reduce(partials[:, b:b + 1], sp_d[:],
                                        mybir.AxisListType.X, ALU.add)
            nc.vector.tensor_reduce(partials[:, NBLK:NBLK + 1], mfT[:],
                                    mybir.AxisListType.X, ALU.add)

            vcol = cp.tile([128, 4], f32, tag="vcol")
            nc.vector.tensor_copy(vcol[:, 0:1], partials[:, NEG:NEG + 1])
            nc.vector.tensor_reduce(vcol[:, 1:2], partials[:, 0:NEG],
                                    mybir.AxisListType.X, ALU.add)
            nc.vector.tensor_copy(vcol[:, 2:3], partials[:, NBLK:NBLK + 1])
            nc.any.memset(vcol[:, 3:4], 0.0)
            ones = cp.tile([128, 1], f32, tag="ones")
            nc.any.memset(ones[:], 1.0)
            psf = pm.tile([128, 2, 512], f32, tag="pm")
            nc.tensor.matmul(psf[0:1, 0, 0:4], ones[:], vcol[:],
                             start=True, stop=True)
            out_sb = cp.tile([1, 4], f32, tag="out_sb")
            nc.scalar.activation(out_sb[:], psf[0:1, 0, 0:4], AF.Copy)
            nc.sync.dma_start(out=d_out[:], in_=out_sb[:])

    nc.finalize()
    return nc


def _get_program(u_list, k_eff, b3f):
    key = (tuple(u_list), k_eff, float(b3f))
    if key not in _PROGRAM_CACHE:
        _PROGRAM_CACHE[key] = _build_program(u_list, k_eff, b3f)
    return _PROGRAM_CACHE[key]


# ------------------------------------------------------------------ kernel

def kernel(**inputs):
    u_list = [int(x) for x in np.asarray(inputs["unroll_subsample"]).reshape(-1)]
    k_eff = max(u_list) + 1
    w = _prep_weights(inputs)
    nc = _get_program(u_list, k_eff, w["b3f"])

    wmaps = {k: v for k, v in w.items() if k != "b3f"}
    in_maps = []
    for c in range(NC):
        m = dict(wmaps)
        m.update(_prep_core(c, inputs, u_list, k_eff))
        in_maps.append(m)

    res = bass_utils.run_bass_kernel_spmd(nc, in_maps, list(range(NC)))
    P = Ng = D = 0.0
    for c in range(NC):
        o = np.asarray(res.results[c]["out"], np.float64)
        P += o[0, 0]
        Ng += o[0, 1]
        D += o[0, 2]
    loss = COEFF * (P / D + Ng / (D * NEG))
    return np.float32(loss)
